# revision 21
# baseline (speedup 1.0000x reference)
"""BlockAttentionResidual Trainium2 kernel (bf16 streaming version).

Math (per token t, feature dim D=1024, over N+1=9 blocks):
    ssq[n,t]  = sum_d v[n,t,d]^2
    rq[n,t]   = (ssq/D + eps)^(-1/2)        (computed as exp(-0.5*ln(ssq/D+eps)))
    logit     = (sum_d w2[d]*v[n,t,d]) * rq      where w2 = proj_w*norm_w
    w[n,t]    = softmax over n of logit
    h[t,d]    = sum_n w[n,t] * v[n,t,d]

Sharding: B*T = 8192 tokens split evenly across 8 cores (1024 tokens/core).

v is converted to bf16 on the host (rel-err budget 2e-2 >> bf16's ~2e-3),
halving HBM read traffic; h is written back as bf16 (cast during the
PSUM->SBUF copy), halving write traffic.

Host-side prep: per core the 9 blocks are pre-interleaved into
vstack[quad, p, (g,d)] where partition p = 14*n + t' stacks the 9 blocks of
14 tokens (126 rows) and the free dim holds 8 such token-groups (two PSUM
pages worth = 112 tokens per "quad"). Each quad's input is a single
contiguous [126, 8192] bf16 DMA with 16KB-per-partition descriptors.

Per-quad on-chip pipeline (engine balance, cost-model cols/quad):
  - ssq:  ScalarE activation(Square, bf16 in/out) with accum_out  (8192 ACT)
  - dot:  VectorE scalar_tensor_tensor mult/mult accum (bf16)     (8192 DVE)
  - softmax over n via TensorE matmuls against a 0/1 mask M[p,t'] = (p%14==t')
      Z = M^T @ exp(logits), and M @ (1/Z) broadcasts 1/Z back to rows.
  - h:    TensorE matmul (bf16 lhsT x bf16 rhs -> f32 PSUM)
      lhsT = M * w_col, 4 groups packed per [128,1024] PSUM page at
      partition offsets 0/32/64/96 (PE column-group tiling).
  - PSUM -> SBUF copy on ScalarE with f32->bf16 cast              (2048 ACT)
  - bf16 DMA out (SWDGE via gpsimd).
"""

import os
import sys
import numpy as np

for _p in ("/opt/trn_rl_repo", "/root/.axon_site/_ro/trn_rl_repo"):
    if os.path.isdir(_p) and _p not in sys.path:
        sys.path.append(_p)

N_CORES = 8
N, B, T, D = 8, 4, 2048, 1024
EPS = 1e-6
TOK = (B * T) // N_CORES          # 1024 tokens per core
NB = N + 1                        # 9 stacked blocks
GROUP = 14                        # tokens per group (14*9 = 126 <= 128)
ROWS = GROUP * NB                 # 126 used partitions
QG = 8                            # groups per quad (two PSUM pages)
QTOK = GROUP * QG                 # 112 tokens per quad
NQUAD = (TOK + QTOK - 1) // QTOK  # 10 (last quad ragged: 16 real tokens)

# Of the two 1024-col PSUM->SBUF page copies per quad, how many go to ACT
# (the rest go to DVE) — balance knob between the two engines.
ACT_COPIES = int(os.environ.get("BLOCKATTN_ACT_COPIES", "1"))
# Every DVE_SQ_EVERY-th quad moves one square-group from ACT to DVE to
# balance the two engines (ACT also carries the Ln/Exp chain + copies).
DVE_SQ_EVERY = int(os.environ.get("BLOCKATTN_DVE_SQ_EVERY", "2"))
ACT_SET = "natural_log_exp_and_others"

_CACHE = {}


def _groups(q):
    """[(g, t0, tg)] active groups of quad q (t0 = core-local token base)."""
    out = []
    for g in range(QG):
        t0 = q * QTOK + g * GROUP
        tg = min(GROUP, TOK - t0)
        if tg > 0:
            out.append((g, t0, tg))
    return out


def _patch_act_tables():
    """Make every activation func this kernel uses resolve to one table set
    (ACT_SET), so bacc emits a single ACT_TABLE_LOAD instead of thrashing
    between sets on every Ln/Exp/Square transition."""
    import concourse.bacc as bacc_mod
    import concourse.hw_specs as hw_specs
    from concourse import mybir

    if getattr(bacc_mod, "_blockattn_act_patch", False):
        return
    AF = mybir.ActivationFunctionType
    mine = {AF.Square, AF.Exp, AF.Ln, AF.Copy, AF.Identity}
    orig = hw_specs.get_activation_tables

    def patched(arch):
        t = dict(orig(arch))
        assert ACT_SET in t and mine <= t[ACT_SET], (ACT_SET, t.get(ACT_SET))
        return {
            name: (funcs if name == ACT_SET else funcs - mine)
            for name, funcs in t.items()
        }

    bacc_mod.get_activation_tables = patched
    bacc_mod._blockattn_act_patch = True


def build_nc():
    import concourse.bacc as bacc
    import concourse.tile as tile
    from concourse import mybir

    _patch_act_tables()

    f32 = mybir.dt.float32
    bf16 = mybir.dt.bfloat16
    AF = mybir.ActivationFunctionType
    OP = mybir.AluOpType

    nc = bacc.Bacc("TRN2", target_bir_lowering=False, debug=False)

    vst_d = nc.dram_tensor("vstack", [NQUAD, ROWS, QG * D], bf16,
                           kind="ExternalInput")
    w2b_d = nc.dram_tensor("w2b", [ROWS, D], bf16, kind="ExternalInput")
    oh_d = nc.dram_tensor("onehot", [ROWS, GROUP], f32, kind="ExternalInput")
    ohT_d = nc.dram_tensor("onehotT", [GROUP, ROWS], f32, kind="ExternalInput")
    oh8_d = nc.dram_tensor("onehot8", [ROWS, QG * GROUP], bf16,
                           kind="ExternalInput")
    h_d = nc.dram_tensor("h", [TOK, D], f32, kind="ExternalOutput")

    vst = vst_d.ap()
    hout = h_d.ap()

    with tile.TileContext(nc) as tc:
        import contextlib
        ctx = contextlib.ExitStack()
        with ctx:
            consts = ctx.enter_context(tc.tile_pool(name="consts", bufs=1))
            vq_pool = ctx.enter_context(tc.tile_pool(name="vq", bufs=4))
            scr_pool = ctx.enter_context(tc.tile_pool(name="scr", bufs=2))
            stats_pool = ctx.enter_context(tc.tile_pool(name="stats", bufs=4))
            small_pool = ctx.enter_context(tc.tile_pool(name="small", bufs=3))
            hsb_pool = ctx.enter_context(tc.tile_pool(name="hsb", bufs=3))
            hpage_pool = ctx.enter_context(
                tc.tile_pool(name="hpage", bufs=3, space="PSUM"))
            zp_pool = ctx.enter_context(
                tc.tile_pool(name="zp", bufs=1, space="PSUM"))
            rzb_pool = ctx.enter_context(
                tc.tile_pool(name="rzb", bufs=1, space="PSUM"))

            # w2b first (the first stt needs it ~1.3 us in); the one-hot
            # masks are only needed at the first zp matmul (~12 us in), so
            # their DMAs are deferred until after quad 0's data is queued.
            w2b = consts.tile([ROWS, D], bf16)
            nc.sync.dma_start(w2b[:], w2b_d.ap()[:])
            oh = consts.tile([ROWS, GROUP], f32)
            ohT = consts.tile([GROUP, ROWS], f32)
            oh8 = consts.tile([ROWS, QG * GROUP], bf16)
            zero_col = consts.tile([ROWS, 1], f32)
            nc.vector.memset(zero_col[:], 0.0)
            eps_col = consts.tile([ROWS, 1], f32)
            nc.vector.memset(eps_col[:], EPS)
            consts_loaded = []

            def load_mask_consts():
                nc.sync.dma_start(oh[:], oh_d.ap()[:])
                nc.sync.dma_start(ohT[:], ohT_d.ap()[:])
                nc.sync.dma_start(oh8[:], oh8_d.ap()[:])
                consts_loaded.append(True)

            # Per-quad live state for the software pipeline:
            # iteration q issues [copies for q-2] [softmax tail for q-1]
            # [DMA + stats + exp chain for q], so each engine's stream has
            # a quad's cross-engine consumers issued 1-2 quads after its
            # producers and never stalls on the in-quad dependency chain.
            state = {}

            def stage_stats(q):
                groups = _groups(q)
                vq = vq_pool.tile([ROWS, QG * D], bf16)
                stats = stats_pool.tile([ROWS, 2 * QG], f32)

                # (tail quad: only transfer the columns of active groups;
                #  first quads: chunked so stats start before the whole
                #  slab lands)
                n_chunks = 4 if q == 0 else (2 if q == 1 else 1)
                if n_chunks > 1:
                    cw = len(groups) * D // n_chunks
                    for ci in range(n_chunks):
                        nc.sync.dma_start(vq[:, ci * cw:(ci + 1) * cw],
                                          vst[q][:, ci * cw:(ci + 1) * cw])
                else:
                    used = len(groups) * D
                    nc.sync.dma_start(vq[:, 0:used], vst[q][:, 0:used])
                if not consts_loaded:
                    load_mask_consts()

                # per-group stats (one full pass each on ACT and DVE; on
                # alternate quads one square moves to DVE to balance ACT's
                # extra load from the Ln/Exp chain and copies)
                n_dve_sq = 1 if (DVE_SQ_EVERY and q % DVE_SQ_EVERY == 0) else 0
                for g, t0, tg in groups:
                    gc = g * D
                    sq_scr = scr_pool.tile([ROWS, D], bf16, tag="sq_scr")
                    if g < n_dve_sq:
                        nc.vector.scalar_tensor_tensor(
                            out=sq_scr[0:ROWS, :], in0=vq[0:ROWS, gc:gc + D],
                            scalar=1.0, in1=vq[0:ROWS, gc:gc + D],
                            op0=OP.mult, op1=OP.mult,
                            accum_out=stats[:, g:g + 1])
                    else:
                        nc.scalar.activation(
                            sq_scr[0:ROWS, :], vq[0:ROWS, gc:gc + D], AF.Square,
                            bias=zero_col[:], accum_out=stats[:, g:g + 1])
                    u_scr = scr_pool.tile([ROWS, D], bf16, tag="u_scr")
                    nc.vector.scalar_tensor_tensor(
                        out=u_scr[0:ROWS, :], in0=vq[0:ROWS, gc:gc + D],
                        scalar=1.0, in1=w2b[0:ROWS, :],
                        op0=OP.mult, op1=OP.mult,
                        accum_out=stats[:, QG + g:QG + g + 1])

                # softmax head on [126, 8] stats: rq, logits, exp, Z matmul
                lnq = small_pool.tile([ROWS, QG], f32, tag="lnq")
                nc.scalar.activation(lnq[:], stats[:, 0:QG], AF.Ln,
                                     bias=eps_col[:], scale=1.0 / D)
                rq = small_pool.tile([ROWS, QG], f32, tag="rq")
                nc.scalar.activation(rq[:], lnq[:], AF.Exp,
                                     bias=zero_col[:], scale=-0.5)
                lg = small_pool.tile([ROWS, QG], f32, tag="lg")
                nc.vector.tensor_mul(lg[:], stats[:, QG:2 * QG], rq[:])
                e_sb = small_pool.tile([ROWS, QG], f32, tag="e_sb")
                nc.scalar.activation(e_sb[:], lg[:], AF.Exp, bias=zero_col[:])

                zp = zp_pool.tile([GROUP, QG], f32)
                nc.tensor.matmul(zp[:], lhsT=oh[:], rhs=e_sb[:],
                                 start=True, stop=True)
                state[q] = {"vq": vq, "e_sb": e_sb, "zp": zp}

            def stage_chain(q):
                """Softmax tail smalls: 1/Z broadcast and masked weights."""
                st = state[q]
                rz = small_pool.tile([GROUP, QG], f32, tag="rz")
                nc.vector.reciprocal(rz[:], st["zp"][:])
                rzb = rzb_pool.tile([ROWS, QG], f32)
                nc.tensor.matmul(rzb[:], lhsT=ohT[:], rhs=rz[:],
                                 start=True, stop=True)
                wcol = small_pool.tile([ROWS, QG], f32, tag="wcol")
                nc.vector.tensor_mul(wcol[:], st["e_sb"][:], rzb[:])
                lhsTs = small_pool.tile([ROWS, QG * GROUP], bf16, tag="lhsTs")
                nc.vector.tensor_tensor(
                    out=lhsTs[:, :].rearrange("p (g j) -> p g j", g=QG),
                    in0=oh8[:, :].rearrange("p (g j) -> p g j", g=QG),
                    in1=wcol[:, :].unsqueeze(2).to_broadcast(
                        [ROWS, QG, GROUP]),
                    op=OP.mult)
                st["lhsTs"] = lhsTs
                st["hpages"] = {}

            def stage_mm(q, pgs, fuse_out=False):
                """Weighted sum via PE for the given PSUM pages (4 groups
                per [128, D] page at partition offsets 0/32/64/96)."""
                st = state[q]
                groups = _groups(q)
                lhsTs, vq = st["lhsTs"], st["vq"]
                active_pages = sorted({g // 4 for g, _, _ in groups})
                for pg in pgs:
                    if pg not in active_pages:
                        continue
                    hpage = hpage_pool.tile([128, D], f32, tag="hpage",
                                            name="hpage")
                    st["hpages"][pg] = hpage
                    for g, t0, tg in groups:
                        if g // 4 != pg:
                            continue
                        gc = g * D
                        lw = lhsTs[:, g * GROUP:(g + 1) * GROUP]
                        col = 32 * (g % 4)
                        for hh in range(2):
                            nc.tensor.matmul(
                                hpage[col:col + GROUP,
                                      512 * hh:512 * hh + 512],
                                lhsT=lw,
                                rhs=vq[0:ROWS,
                                       gc + 512 * hh:gc + 512 * hh + 512],
                                start=True, stop=True,
                                tile_position=(0, col))
                    if fuse_out:
                        # pipeline drain: copy + store this page right away
                        # (no later stats left to hide the latency behind)
                        _page_out(q, groups, hpage, pg, pg % 2 < ACT_COPIES)

            def _page_out(q, groups, hpage, pg, on_act):
                h_sb = hsb_pool.tile([128, D], f32, tag="h_sb")
                if on_act:
                    nc.scalar.copy(h_sb[:], hpage[:])
                else:
                    nc.vector.tensor_copy(h_sb[:], hpage[:])
                for g, t0, tg in groups:
                    if g // 4 != pg:
                        continue
                    nc.gpsimd.dma_start(
                        hout[t0:t0 + tg, :],
                        h_sb[32 * (g % 4):32 * (g % 4) + tg, :])

            def stage_out(q):
                st = state.pop(q)
                groups = _groups(q)
                for pi, pg in enumerate(sorted(st["hpages"])):
                    _page_out(q, groups, st["hpages"][pg], pg,
                              pi < ACT_COPIES)

            # Steady state defers a quad's PE matmuls one iteration and its
            # copies two, so stats always have the whole engine. The last
            # full quad (SL) starts its tail early (page 0 at its own
            # iteration, page 1 the next) so its matmuls run under the tail
            # quad's stats and the drain only exposes the tiny last quad.
            SL = NQUAD - 2
            for q in range(NQUAD):
                if q - 2 >= 0:
                    stage_out(q - 2)
                if q - 1 >= 0:
                    if q - 1 == SL:
                        stage_mm(SL, [1])
                    else:
                        stage_chain(q - 1)
                        stage_mm(q - 1, [0, 1])
                stage_stats(q)
                if q == SL:
                    stage_chain(SL)
                    stage_mm(SL, [0])
            stage_out(SL)
            stage_chain(NQUAD - 1)
            stage_mm(NQUAD - 1, [0, 1], fuse_out=True)
            state.pop(NQUAD - 1)

    nc.compile()
    return nc


def _host_inputs(blocks, partial_block, proj_w, norm_w):
    """Slice + interleave per-core inputs (host-side, numpy only)."""
    import ml_dtypes
    bf16 = ml_dtypes.bfloat16

    blocks = np.asarray(blocks, np.float32).reshape(N, B * T, D)
    partial = np.asarray(partial_block, np.float32).reshape(B * T, D)
    w2 = (np.asarray(proj_w, np.float32) * np.asarray(norm_w, np.float32))
    w2b = np.ascontiguousarray(
        np.broadcast_to(w2.astype(bf16), (ROWS, D)))
    oh = np.zeros((ROWS, GROUP), np.float32)
    for p in range(ROWS):
        oh[p, p % GROUP] = 1.0
    ohT = np.ascontiguousarray(oh.T)
    oh8 = np.ascontiguousarray(np.tile(oh, (1, QG))).astype(bf16)

    pad_tok = NQUAD * QTOK  # 1120
    in_maps = []
    for c in range(N_CORES):
        s = slice(c * TOK, (c + 1) * TOK)
        av = np.zeros((NB, pad_tok, D), bf16)
        av[:N, :TOK] = blocks[:, s, :].astype(bf16)
        av[N, :TOK] = partial[s, :].astype(bf16)
        # vstack[q, 14n+t', g*D+d] = av[n, q*112 + g*14 + t', d]
        vst = av.reshape(NB, NQUAD, QG, GROUP, D)
        vst = np.ascontiguousarray(vst.transpose(1, 0, 3, 2, 4))
        vst = vst.reshape(NQUAD, ROWS, QG * D)
        in_maps.append({
            "vstack": vst,
            "w2b": w2b,
            "onehot": oh,
            "onehotT": ohT,
            "onehot8": oh8,
        })
    return in_maps


def kernel(blocks, partial_block, proj_w, norm_w):
    from concourse.bass_utils import run_bass_kernel_spmd

    if "nc" not in _CACHE:
        _CACHE["nc"] = build_nc()
    nc = _CACHE["nc"]
    in_maps = _host_inputs(blocks, partial_block, proj_w, norm_w)
    res = run_bass_kernel_spmd(nc, in_maps, core_ids=list(range(N_CORES)))
    h = np.concatenate([res.results[c]["h"] for c in range(N_CORES)], axis=0)
    return np.asarray(h, dtype=np.float32).reshape(B, T, D)


# revision 23
# speedup vs baseline: 1.0138x; 1.0138x over previous
"""BlockAttentionResidual Trainium2 kernel (bf16 streaming version).

Math (per token t, feature dim D=1024, over N+1=9 blocks):
    ssq[n,t]  = sum_d v[n,t,d]^2
    rq[n,t]   = (ssq/D + eps)^(-1/2)        (computed as exp(-0.5*ln(ssq/D+eps)))
    logit     = (sum_d w2[d]*v[n,t,d]) * rq      where w2 = proj_w*norm_w
    w[n,t]    = softmax over n of logit
    h[t,d]    = sum_n w[n,t] * v[n,t,d]

Sharding: B*T = 8192 tokens split evenly across 8 cores (1024 tokens/core).

v is converted to bf16 on the host (rel-err budget 2e-2 >> bf16's ~2e-3),
halving HBM read traffic; h is written back as bf16 (cast during the
PSUM->SBUF copy), halving write traffic.

Host-side prep: per core the 9 blocks are pre-interleaved into
vstack[quad, p, (g,d)] where partition p = 14*n + t' stacks the 9 blocks of
14 tokens (126 rows) and the free dim holds 8 such token-groups (two PSUM
pages worth = 112 tokens per "quad"). Each quad's input is a single
contiguous [126, 8192] bf16 DMA with 16KB-per-partition descriptors.

Per-quad on-chip pipeline (engine balance, cost-model cols/quad):
  - ssq:  ScalarE activation(Square, bf16 in/out) with accum_out  (8192 ACT)
  - dot:  VectorE scalar_tensor_tensor mult/mult accum (bf16)     (8192 DVE)
  - softmax over n via TensorE matmuls against a 0/1 mask M[p,t'] = (p%14==t')
      Z = M^T @ exp(logits), and M @ (1/Z) broadcasts 1/Z back to rows.
  - h:    TensorE matmul (bf16 lhsT x bf16 rhs -> f32 PSUM)
      lhsT = M * w_col, 4 groups packed per [128,1024] PSUM page at
      partition offsets 0/32/64/96 (PE column-group tiling).
  - PSUM -> SBUF copy on ScalarE with f32->bf16 cast              (2048 ACT)
  - bf16 DMA out (SWDGE via gpsimd).
"""

import os
import sys
import numpy as np

for _p in ("/opt/trn_rl_repo", "/root/.axon_site/_ro/trn_rl_repo"):
    if os.path.isdir(_p) and _p not in sys.path:
        sys.path.append(_p)

N_CORES = 8
N, B, T, D = 8, 4, 2048, 1024
EPS = 1e-6
TOK = (B * T) // N_CORES          # 1024 tokens per core
NB = N + 1                        # 9 stacked blocks
GROUP = 14                        # tokens per group (14*9 = 126 <= 128)
ROWS = GROUP * NB                 # 126 used partitions
QG = 8                            # groups per quad (two PSUM pages)
QTOK = GROUP * QG                 # 112 tokens per quad
NQUAD = (TOK + QTOK - 1) // QTOK  # 10 (last quad ragged: 16 real tokens)

# Of the two 1024-col PSUM->SBUF page copies per quad, how many go to ACT
# (the rest go to DVE) — balance knob between the two engines.
ACT_COPIES = int(os.environ.get("BLOCKATTN_ACT_COPIES", "1"))
# Every DVE_SQ_EVERY-th quad moves one square-group from ACT to DVE to
# balance the two engines (ACT also carries the Ln/Exp chain + copies).
DVE_SQ_EVERY = int(os.environ.get("BLOCKATTN_DVE_SQ_EVERY", "2"))
ACT_SET = "natural_log_exp_and_others"

_CACHE = {}


def _groups(q):
    """[(g, t0, tg)] active groups of quad q (t0 = core-local token base)."""
    out = []
    for g in range(QG):
        t0 = q * QTOK + g * GROUP
        tg = min(GROUP, TOK - t0)
        if tg > 0:
            out.append((g, t0, tg))
    return out


def _patch_act_tables():
    """Make every activation func this kernel uses resolve to one table set
    (ACT_SET), so bacc emits a single ACT_TABLE_LOAD instead of thrashing
    between sets on every Ln/Exp/Square transition."""
    import concourse.bacc as bacc_mod
    import concourse.hw_specs as hw_specs
    from concourse import mybir

    if getattr(bacc_mod, "_blockattn_act_patch", False):
        return
    AF = mybir.ActivationFunctionType
    mine = {AF.Square, AF.Exp, AF.Ln, AF.Copy, AF.Identity}
    orig = hw_specs.get_activation_tables

    def patched(arch):
        t = dict(orig(arch))
        assert ACT_SET in t and mine <= t[ACT_SET], (ACT_SET, t.get(ACT_SET))
        return {
            name: (funcs if name == ACT_SET else funcs - mine)
            for name, funcs in t.items()
        }

    bacc_mod.get_activation_tables = patched
    bacc_mod._blockattn_act_patch = True


def build_nc():
    import concourse.bacc as bacc
    import concourse.tile as tile
    from concourse import mybir

    _patch_act_tables()

    f32 = mybir.dt.float32
    bf16 = mybir.dt.bfloat16
    AF = mybir.ActivationFunctionType
    OP = mybir.AluOpType

    nc = bacc.Bacc("TRN2", target_bir_lowering=False, debug=False)

    vst_d = nc.dram_tensor("vstack", [NQUAD, ROWS, QG * D], bf16,
                           kind="ExternalInput")
    w2b_d = nc.dram_tensor("w2b", [ROWS, D], bf16, kind="ExternalInput")
    oh_d = nc.dram_tensor("onehot", [ROWS, GROUP], f32, kind="ExternalInput")
    ohT_d = nc.dram_tensor("onehotT", [GROUP, ROWS], f32, kind="ExternalInput")
    oh8_d = nc.dram_tensor("onehot8", [ROWS, QG * GROUP], bf16,
                           kind="ExternalInput")
    h_d = nc.dram_tensor("h", [TOK, D], f32, kind="ExternalOutput")

    vst = vst_d.ap()
    hout = h_d.ap()

    with tile.TileContext(nc) as tc:
        import contextlib
        ctx = contextlib.ExitStack()
        with ctx:
            consts = ctx.enter_context(tc.tile_pool(name="consts", bufs=1))
            vq_pool = ctx.enter_context(tc.tile_pool(name="vq", bufs=4))
            scr_pool = ctx.enter_context(tc.tile_pool(name="scr", bufs=2))
            stats_pool = ctx.enter_context(tc.tile_pool(name="stats", bufs=4))
            small_pool = ctx.enter_context(tc.tile_pool(name="small", bufs=3))
            hsb_pool = ctx.enter_context(tc.tile_pool(name="hsb", bufs=3))
            hpage_pool = ctx.enter_context(
                tc.tile_pool(name="hpage", bufs=3, space="PSUM"))
            zp_pool = ctx.enter_context(
                tc.tile_pool(name="zp", bufs=1, space="PSUM"))
            rzb_pool = ctx.enter_context(
                tc.tile_pool(name="rzb", bufs=1, space="PSUM"))

            # w2b first (the first stt needs it ~1.3 us in); the one-hot
            # masks are only needed at the first zp matmul (~12 us in), so
            # their DMAs are deferred until after quad 0's data is queued.
            w2b = consts.tile([ROWS, D], bf16)
            nc.sync.dma_start(w2b[:], w2b_d.ap()[:])
            oh = consts.tile([ROWS, GROUP], f32)
            ohT = consts.tile([GROUP, ROWS], f32)
            oh8 = consts.tile([ROWS, QG * GROUP], bf16)
            zero_col = consts.tile([ROWS, 1], f32)
            nc.vector.memset(zero_col[:], 0.0)
            eps_col = consts.tile([ROWS, 1], f32)
            nc.vector.memset(eps_col[:], EPS)
            consts_loaded = []

            def load_mask_consts():
                nc.sync.dma_start(oh[:], oh_d.ap()[:])
                nc.sync.dma_start(ohT[:], ohT_d.ap()[:])
                nc.sync.dma_start(oh8[:], oh8_d.ap()[:])
                consts_loaded.append(True)

            # Per-quad live state for the software pipeline:
            # iteration q issues [copies for q-2] [softmax tail for q-1]
            # [DMA + stats + exp chain for q], so each engine's stream has
            # a quad's cross-engine consumers issued 1-2 quads after its
            # producers and never stalls on the in-quad dependency chain.
            state = {}

            def stage_dma(q):
                """Prefetch quad q's input slab (issued one iteration
                ahead of its stats). Tail quad: only the active groups'
                columns; first quads: chunked so stats can start before
                the whole slab lands."""
                groups = _groups(q)
                vq = vq_pool.tile([ROWS, QG * D], bf16)
                n_chunks = 4 if q == 0 else (2 if q == 1 else 1)
                if n_chunks > 1:
                    cw = len(groups) * D // n_chunks
                    for ci in range(n_chunks):
                        nc.sync.dma_start(vq[:, ci * cw:(ci + 1) * cw],
                                          vst[q][:, ci * cw:(ci + 1) * cw])
                else:
                    used = len(groups) * D
                    nc.sync.dma_start(vq[:, 0:used], vst[q][:, 0:used])
                state[q] = {"vq": vq}

            def stage_stats(q):
                groups = _groups(q)
                vq = state[q]["vq"]
                stats = stats_pool.tile([ROWS, 2 * QG], f32)

                # per-group stats (one full pass each on ACT and DVE; on
                # alternate quads one square moves to DVE to balance ACT's
                # extra load from the Ln/Exp chain and copies)
                n_dve_sq = 1 if (DVE_SQ_EVERY and q % DVE_SQ_EVERY == 0) else 0
                for g, t0, tg in groups:
                    gc = g * D
                    sq_scr = scr_pool.tile([ROWS, D], bf16, tag="sq_scr")
                    if g < n_dve_sq:
                        nc.vector.scalar_tensor_tensor(
                            out=sq_scr[0:ROWS, :], in0=vq[0:ROWS, gc:gc + D],
                            scalar=1.0, in1=vq[0:ROWS, gc:gc + D],
                            op0=OP.mult, op1=OP.mult,
                            accum_out=stats[:, g:g + 1])
                    else:
                        nc.scalar.activation(
                            sq_scr[0:ROWS, :], vq[0:ROWS, gc:gc + D], AF.Square,
                            bias=zero_col[:], accum_out=stats[:, g:g + 1])
                    u_scr = scr_pool.tile([ROWS, D], bf16, tag="u_scr")
                    nc.vector.scalar_tensor_tensor(
                        out=u_scr[0:ROWS, :], in0=vq[0:ROWS, gc:gc + D],
                        scalar=1.0, in1=w2b[0:ROWS, :],
                        op0=OP.mult, op1=OP.mult,
                        accum_out=stats[:, QG + g:QG + g + 1])

                # softmax head on [126, 8] stats: rq, logits, exp, Z matmul
                lnq = small_pool.tile([ROWS, QG], f32, tag="lnq")
                nc.scalar.activation(lnq[:], stats[:, 0:QG], AF.Ln,
                                     bias=eps_col[:], scale=1.0 / D)
                rq = small_pool.tile([ROWS, QG], f32, tag="rq")
                nc.scalar.activation(rq[:], lnq[:], AF.Exp,
                                     bias=zero_col[:], scale=-0.5)
                lg = small_pool.tile([ROWS, QG], f32, tag="lg")
                nc.vector.tensor_mul(lg[:], stats[:, QG:2 * QG], rq[:])
                e_sb = small_pool.tile([ROWS, QG], f32, tag="e_sb")
                nc.scalar.activation(e_sb[:], lg[:], AF.Exp, bias=zero_col[:])

                state[q]["e_sb"] = e_sb

            def stage_chain(q):
                """Softmax tail: Z-sum matmul, 1/Z broadcast, masked
                weights."""
                st = state[q]
                zp = zp_pool.tile([GROUP, QG], f32)
                nc.tensor.matmul(zp[:], lhsT=oh[:], rhs=st["e_sb"][:],
                                 start=True, stop=True)
                rz = small_pool.tile([GROUP, QG], f32, tag="rz")
                nc.vector.reciprocal(rz[:], zp[:])
                rzb = rzb_pool.tile([ROWS, QG], f32)
                nc.tensor.matmul(rzb[:], lhsT=ohT[:], rhs=rz[:],
                                 start=True, stop=True)
                wcol = small_pool.tile([ROWS, QG], f32, tag="wcol")
                nc.vector.tensor_mul(wcol[:], st["e_sb"][:], rzb[:])
                lhsTs = small_pool.tile([ROWS, QG * GROUP], bf16, tag="lhsTs")
                nc.vector.tensor_tensor(
                    out=lhsTs[:, :].rearrange("p (g j) -> p g j", g=QG),
                    in0=oh8[:, :].rearrange("p (g j) -> p g j", g=QG),
                    in1=wcol[:, :].unsqueeze(2).to_broadcast(
                        [ROWS, QG, GROUP]),
                    op=OP.mult)
                st["lhsTs"] = lhsTs
                st["hpages"] = {}

            def stage_mm(q, pgs, fuse_out=False):
                """Weighted sum via PE for the given PSUM pages (4 groups
                per [128, D] page at partition offsets 0/32/64/96)."""
                st = state[q]
                groups = _groups(q)
                lhsTs, vq = st["lhsTs"], st["vq"]
                active_pages = sorted({g // 4 for g, _, _ in groups})
                for pg in pgs:
                    if pg not in active_pages:
                        continue
                    hpage = hpage_pool.tile([128, D], f32, tag="hpage",
                                            name="hpage")
                    st["hpages"][pg] = hpage
                    for g, t0, tg in groups:
                        if g // 4 != pg:
                            continue
                        gc = g * D
                        lw = lhsTs[:, g * GROUP:(g + 1) * GROUP]
                        col = 32 * (g % 4)
                        for hh in range(2):
                            nc.tensor.matmul(
                                hpage[col:col + GROUP,
                                      512 * hh:512 * hh + 512],
                                lhsT=lw,
                                rhs=vq[0:ROWS,
                                       gc + 512 * hh:gc + 512 * hh + 512],
                                start=True, stop=True,
                                tile_position=(0, col))
                    if fuse_out:
                        # pipeline drain: copy + store this page right away
                        # (no later stats left to hide the latency behind)
                        _page_out(q, groups, hpage, pg, pg % 2 < ACT_COPIES)

            def _page_out(q, groups, hpage, pg, on_act):
                h_sb = hsb_pool.tile([128, D], f32, tag="h_sb")
                if on_act:
                    nc.scalar.copy(h_sb[:], hpage[:])
                else:
                    nc.vector.tensor_copy(h_sb[:], hpage[:])
                # During the drain the ALUs are idle, so spread the last
                # quads' store-descriptor generation across three queues
                # instead of serializing ~630ns/DMA on the Q7.
                drain = q >= NQUAD - 3
                engs = ([nc.gpsimd, nc.scalar, nc.sync] if drain
                        else [nc.gpsimd])
                for i, (g, t0, tg) in enumerate(
                        [gt for gt in groups if gt[0] // 4 == pg]):
                    engs[i % len(engs)].dma_start(
                        hout[t0:t0 + tg, :],
                        h_sb[32 * (g % 4):32 * (g % 4) + tg, :])

            def stage_out(q):
                st = state.pop(q)
                groups = _groups(q)
                for pi, pg in enumerate(sorted(st["hpages"])):
                    _page_out(q, groups, st["hpages"][pg], pg,
                              pi < ACT_COPIES)

            # Steady state defers a quad's PE matmuls one iteration and its
            # copies two, so stats always have the whole engine. The last
            # full quad (SL) starts its tail early (page 0 at its own
            # iteration, page 1 the next) so its matmuls run under the tail
            # quad's stats and the drain only exposes the tiny last quad.
            SL = NQUAD - 2
            stage_dma(0)
            stage_dma(1)
            for q in range(NQUAD):
                if q == 1:
                    load_mask_consts()
                if q + 2 < NQUAD:
                    stage_dma(q + 2)
                if q - 2 >= 0:
                    stage_out(q - 2)
                if q - 1 >= 0:
                    if q - 1 == SL:
                        stage_mm(SL, [1])
                    else:
                        stage_chain(q - 1)
                        stage_mm(q - 1, [0, 1])
                stage_stats(q)
                if q == SL:
                    stage_chain(SL)
                    stage_mm(SL, [0])
            stage_out(SL)
            stage_chain(NQUAD - 1)
            stage_mm(NQUAD - 1, [0, 1], fuse_out=True)
            state.pop(NQUAD - 1)

    nc.compile()
    return nc


def _host_inputs(blocks, partial_block, proj_w, norm_w):
    """Slice + interleave per-core inputs (host-side, numpy only)."""
    import ml_dtypes
    bf16 = ml_dtypes.bfloat16

    blocks = np.asarray(blocks, np.float32).reshape(N, B * T, D)
    partial = np.asarray(partial_block, np.float32).reshape(B * T, D)
    w2 = (np.asarray(proj_w, np.float32) * np.asarray(norm_w, np.float32))
    w2b = np.ascontiguousarray(
        np.broadcast_to(w2.astype(bf16), (ROWS, D)))
    oh = np.zeros((ROWS, GROUP), np.float32)
    for p in range(ROWS):
        oh[p, p % GROUP] = 1.0
    ohT = np.ascontiguousarray(oh.T)
    oh8 = np.ascontiguousarray(np.tile(oh, (1, QG))).astype(bf16)

    pad_tok = NQUAD * QTOK  # 1120
    in_maps = []
    for c in range(N_CORES):
        s = slice(c * TOK, (c + 1) * TOK)
        av = np.zeros((NB, pad_tok, D), bf16)
        av[:N, :TOK] = blocks[:, s, :].astype(bf16)
        av[N, :TOK] = partial[s, :].astype(bf16)
        # vstack[q, 14n+t', g*D+d] = av[n, q*112 + g*14 + t', d]
        vst = av.reshape(NB, NQUAD, QG, GROUP, D)
        vst = np.ascontiguousarray(vst.transpose(1, 0, 3, 2, 4))
        vst = vst.reshape(NQUAD, ROWS, QG * D)
        in_maps.append({
            "vstack": vst,
            "w2b": w2b,
            "onehot": oh,
            "onehotT": ohT,
            "onehot8": oh8,
        })
    return in_maps


def kernel(blocks, partial_block, proj_w, norm_w):
    from concourse.bass_utils import run_bass_kernel_spmd

    if "nc" not in _CACHE:
        _CACHE["nc"] = build_nc()
    nc = _CACHE["nc"]
    in_maps = _host_inputs(blocks, partial_block, proj_w, norm_w)
    res = run_bass_kernel_spmd(nc, in_maps, core_ids=list(range(N_CORES)))
    h = np.concatenate([res.results[c]["h"] for c in range(N_CORES)], axis=0)
    return np.asarray(h, dtype=np.float32).reshape(B, T, D)
